# revision 19
# baseline (speedup 1.0000x reference)
"""Trainium2 Bass kernel for BaselineDNN pooling problem.

Per core (512 of 4096 batch rows, data-parallel across 8 cores):
  1. dma_gather (InstDMAGatherAnt ucode) fetches embedding rows from a
     per-group host-compacted table ([~23k unique rows, 768B stride])
     into [128 rows, 8 tokens, EP] SBUF tiles: index list position
     i = t*128 + p -> partition p, slot t. 1024 rows per instruction
     (int16 indices, 16-partition-wrapped, replicated x8), rotating over
     4 SWDGE queues so desc-gen overlaps DMA drain.
  2. PE identity-matmul accumulates the token sum into PSUM -> mean
     pool (x 1/len on ACT)
  3. DVE TT-max chain over valid chunks only (rows length-sorted on
     host so each 128-row group has a tight valid band). Boundary
     tokens are masked by per-token ACT bias-adds (+0 / -1e30): the
     Scalar engine is nearly idle, and a DVE broadcast add would run
     in slow 1x mode.
  4. PE transposes rep ([128,600] -> 5 x [120,128]) into rep_T
  5. MLP on PE (h_T = relu(W1_T @ rep_T + b1), out_T = W2_T @ h_T + b2)
  6. out_T [3,512] DMA'd out; host inverts the row permutation.

Scheduling: groups processed longest-first (3,2,1,0) so the final
group's tail is short; each group's tail work is EMITTED in two phases
during the next group's chunk stream (folds+mean+transposes, then MLP)
so the serial DVE fold chain doesn't head-of-line-block the pair ops
that recycle gather buffers. The identity matrix ships as a DRAM input
(generating it with Pool-ALU ucode would force the Pool engine through
a ucode-overlay switch costing ~12us before the first gather). First
group's indices load via SWDGE on the gpsimd engine itself; weight
DMAs are emitted mid-stream.

Self-contained: hardcodes all shapes from the problem spec.
"""

import numpy as np
from contextlib import ExitStack

import ml_dtypes

import concourse.bacc as bacc
import concourse.tile as tile
from concourse import mybir
from concourse.bass_utils import run_bass_kernel_spmd

VOCAB, DIM = 100000, 300
B, L = 4096, 200
HIDDEN, OUT = 1000, 3
NCORES = 8
P = 128
RPC = B // NCORES            # 512 rows per core
G = RPC // P                 # 4 groups of 128 rows
TC = 8                       # tokens per gather chunk (NI=1024 HW limit)
NCH = L // TC                # 25 chunks
NI = P * TC                  # 1024 indices per gather
CW = NI // 16                # idx columns per chunk (16-partition wrap)
NEG = -1.0e30
NQ = 4                       # SWDGE queues

EP = 384                     # padded row length (768B, 256B-divisible)

KC = 120                     # rep contraction chunk (600 = 5*120)
NKC = (2 * DIM) // KC        # 5
MJ = 125                     # hidden m-chunk (1000 = 8*125)
NJ = HIDDEN // MJ            # 8

GROUP_ORDER = (3, 2, 1, 0)   # longest valid window first
TAIL_A = 9                   # prev group: folds+mean+transposes
TAIL_B = 13                  # prev group: MLP
W_AT = 12                    # weight-DMA emission point in first group
LAST_A1 = 14                 # last group: early max-side emission point

F32 = mybir.dt.float32
BF16 = mybir.dt.bfloat16
GDT = BF16
GNP = ml_dtypes.bfloat16
I16 = mybir.dt.int16
AX = mybir.AxisListType
ALU = mybir.AluOpType
ACT_F = mybir.ActivationFunctionType

XGW = NCH * CW               # 1600 idx columns per group

_BUILD_CACHE = {}


def _build(lhi, llo, vg):
    """Emit the SPMD program. lhi/llo: per-group max/min valid length;
    vg: padded per-group compact-table row count (identical across cores
    by construction)."""
    nc = bacc.Bacc(
        "TRN2", target_bir_lowering=False, debug=False, enable_asserts=False,
        num_swdge_queues=NQ, dynamic_dma_scratch_size=32768,
    )
    gtab = nc.dram_tensor("gtab", [G, vg, EP], GDT, kind="ExternalInput")
    # index columns packed in device group order (GROUP_ORDER[0] first)
    xg = nc.dram_tensor("xg", [P, G * XGW], I16, kind="ExternalInput")
    aoff = nc.dram_tensor("aoff", [P, G * L], F32, kind="ExternalInput")
    invlen = nc.dram_tensor("invlen", [P, G], F32, kind="ExternalInput")
    identD = nc.dram_tensor("identD", [P, P], F32, kind="ExternalInput")
    w1 = nc.dram_tensor("w1", [2 * DIM, HIDDEN], BF16, kind="ExternalInput")
    b1 = nc.dram_tensor("b1", [HIDDEN], F32, kind="ExternalInput")
    w2 = nc.dram_tensor("w2", [HIDDEN, OUT], BF16, kind="ExternalInput")
    b2 = nc.dram_tensor("b2", [OUT], F32, kind="ExternalInput")
    out_t = nc.dram_tensor("out_t", [OUT, RPC], F32, kind="ExternalOutput")

    with tile.TileContext(nc) as tc, ExitStack() as ctx:
        persist = ctx.enter_context(tc.tile_pool(name="persist", bufs=1))
        gpool = ctx.enter_context(tc.tile_pool(name="gpool", bufs=10))
        xpool = ctx.enter_context(tc.tile_pool(name="xpool", bufs=7))
        mpool = ctx.enter_context(tc.tile_pool(name="mpool", bufs=2))
        spsum = ctx.enter_context(tc.tile_pool(name="spsum", bufs=2, space="PSUM"))
        ppool = ctx.enter_context(tc.tile_pool(name="ppool", bufs=2, space="PSUM"))
        hpool = ctx.enter_context(tc.tile_pool(name="hpool", bufs=2, space="PSUM"))
        opool = ctx.enter_context(tc.tile_pool(name="opool", bufs=1, space="PSUM"))

        # first group's indices via SWDGE on the gather engine itself —
        # no cross-engine semaphores before the first gather
        xo0 = persist.tile([P, XGW], I16, tag="xo0", name="xo0")
        nc.gpsimd.dma_start(xo0[:], xg[:, 0:XGW])

        ident = persist.tile([P, P], F32, tag="ident")
        nc.sync.dma_start(ident[:], identD[:])
        ident_bf = persist.tile([P, P], BF16, tag="ident_bf")
        nc.scalar.copy(out=ident_bf[:], in_=ident[:])

        # small per-group inputs (single consolidated DMAs)
        ao_all = persist.tile([P, G * L], F32, tag="ao_all", name="ao_all")
        nc.sync.dma_start(ao_all[:], aoff[:])
        il_all = persist.tile([P, G], F32, tag="il_all", name="il_all")
        nc.sync.dma_start(il_all[:], invlen[:])

        # weight tiles (DMAs emitted mid-stream at W_AT of first group)
        w1_t = [persist.tile([KC, HIDDEN], BF16, tag=f"w1_{k}", name=f"w1_{k}")
                for k in range(NKC)]
        w2_t = [persist.tile([MJ, OUT], BF16, tag=f"w2_{j}", name=f"w2_{j}")
                for j in range(NJ)]
        b1_t = [persist.tile([MJ, 1], F32, tag=f"b1_{j}", name=f"b1_{j}")
                for j in range(NJ)]
        b2_t = persist.tile([OUT, 1], F32, tag="b2")
        xo_rest = persist.tile([P, (G - 1) * XGW], I16, tag="xo_r", name="xo_r")

        def emit_weight_dmas():
            for k in range(NKC):
                nc.sync.dma_start(w1_t[k][:], w1[k * KC:(k + 1) * KC, :])
            for j in range(NJ):
                nc.sync.dma_start(w2_t[j][:], w2[j * MJ:(j + 1) * MJ, :])
                nc.sync.dma_start(b1_t[j][:], b1[j * MJ:(j + 1) * MJ, None])
            nc.sync.dma_start(b2_t[:], b2[:, None])
            nc.sync.dma_start(xo_rest[:], xg[:, XGW:])

        rep_t = [persist.tile([KC, RPC], BF16, tag=f"repT_{k}", name=f"repT_{k}")
                 for k in range(NKC)]
        h_t = [persist.tile([MJ, RPC], BF16, tag=f"hT_{j}", name=f"hT_{j}")
               for j in range(NJ)]
        ot_sb = persist.tile([OUT, RPC], F32, tag="ot", name="ot")

        def ap3(t):
            """[p, t, 0:DIM] view of a [P, TC*EP] chunk tile."""
            return t[:].rearrange("p (t e) -> p t e", e=EP)[:, :, 0:DIM]

        def ap3c(t):
            """[p, t, d] view of a [P, TC*DIM] contiguous tile."""
            return t[:].rearrange("p (t d) -> p t d", d=DIM)

        def fold(stack, op, pool, tag):
            # collapse remaining binary-counter levels into one root
            nodes = [stack[lv] for lv in sorted(stack)]
            stack.clear()
            while len(nodes) > 1:
                t = pool.tile([P, TC * DIM], GDT, tag=tag, name=tag)
                nc.vector.tensor_tensor(
                    out=ap3c(t), in0=nodes[0], in1=nodes[1], op=op)
                nodes = [ap3c(t)] + nodes[2:]
            return nodes[0]

        def push(stack, node, op, pool, tag):
            lv = 0
            while lv in stack:
                other = stack.pop(lv)
                t = pool.tile([P, TC * DIM], GDT, tag=tag, name=tag)
                nc.vector.tensor_tensor(
                    out=ap3c(t), in0=other, in1=node, op=op)
                node = ap3c(t)
                lv += 1
            stack[lv] = node

        def tfold_max(root, pool, out_f32):
            # fold token slots 8->4->2->1 into the f32 rep slice
            cur, nt = root, TC
            while nt > 2:
                nt //= 2
                t = pool.tile([P, nt * DIM], GDT, tag=f"tmf{nt}",
                              name="tmf", bufs=2)
                nc.vector.tensor_tensor(
                    out=t[:].rearrange("p (t d) -> p t d", d=DIM),
                    in0=cur[:, 0:nt, :], in1=cur[:, nt:2 * nt, :], op=ALU.max)
                cur = t[:].rearrange("p (t d) -> p t d", d=DIM)
            nc.vector.tensor_tensor(
                out=out_f32, in0=cur[:, 0:1, :],
                in1=cur[:, 1:2, :], op=ALU.max)

        ni_reg = NI
        qn = 0
        tails = []   # pending (phaseA, phaseB) closures

        def make_tail(g, max_stack, ps):
            gsl = slice(g * P, (g + 1) * P)
            il = il_all[:, g:g + 1]
            rep_sb = mpool.tile([P, 2 * DIM], F32, tag="rep_sb", name="rep_sb")

            def trans_rep(k):
                pt = ppool.tile([KC, P], F32, tag="pt", name="pt")
                nc.tensor.transpose(
                    out=pt[:], in_=rep_sb[:, k * KC:(k + 1) * KC],
                    identity=ident[:],
                )
                nc.scalar.copy(out=rep_t[k][:, gsl], in_=pt[:])

            def max_part():
                max_root = fold(max_stack, ALU.max, xpool, "tm")
                tfold_max(max_root, xpool,
                          rep_sb[:, DIM:2 * DIM].unsqueeze(1))

            def mean_part():
                nc.scalar.mul(rep_sb[:, 0:DIM], ps[:], il)

            def phase_a():
                max_part()
                mean_part()
                for k in range(NKC):
                    trans_rep(k)

            def mlp(hps, klist, start, stop):
                for j in range(NJ):
                    for ki, k in enumerate(klist):
                        nc.tensor.matmul(
                            out=hps[j][:],
                            lhsT=w1_t[k][:, j * MJ:(j + 1) * MJ],
                            rhs=rep_t[k][:, gsl],
                            start=(start and ki == 0),
                            stop=(stop and ki == len(klist) - 1),
                        )

            def finish(hps):
                for j in range(NJ):
                    nc.scalar.activation(
                        out=h_t[j][:, gsl], in_=hps[j][:], func=ACT_F.Relu,
                        bias=b1_t[j][:, 0:1], scale=1.0,
                    )
                op_ps = opool.tile([OUT, P], F32, tag="op", name="op", bufs=2)
                for j in range(NJ):
                    nc.tensor.matmul(
                        out=op_ps[:], lhsT=w2_t[j][:], rhs=h_t[j][:, gsl],
                        start=(j == 0), stop=(j == NJ - 1),
                    )
                nc.scalar.activation(
                    out=ot_sb[:, gsl], in_=op_ps[:], func=ACT_F.Identity,
                    bias=b2_t[:, 0:1], scale=1.0,
                )
                nc.sync.dma_start(out_t[:, gsl], ot_sb[:, gsl])

            def phase_b():
                hps = [hpool.tile([MJ, P], F32, tag="hp", name="hp")
                       for _ in range(NJ)]
                mlp(hps, list(range(NKC)), True, True)
                finish(hps)

            # split variant for the last group: max side early, mean late
            def early_max():
                # max dims are rep rows 300:600 -> k-chunks 3,4 pure max
                max_part()
                trans_rep(3)
                trans_rep(4)
                return None

            def late_mean(_):
                mean_part()
                trans_rep(0)
                trans_rep(1)
                trans_rep(2)
                phase_b()

            return phase_a, phase_b, early_max, late_mean

        for gi, g in enumerate(GROUP_ORDER):
            xcol0 = (gi - 1) * XGW

            nv = -(-lhi[g] // TC)          # chunks partaking in max pool
            mhi = min(nv * TC, L)          # mask window end (chunk-rounded)

            max_stack = {}
            ps = spsum.tile([P, DIM], F32, tag="ps", name="ps")
            cur_tail = make_tail(g, max_stack, ps)

            gtiles = []
            for c in range(NCH):
                gt = gpool.tile([P, TC * EP], GDT, tag="gt", name="gt")
                idx_ap = (xo0[:, c * CW:(c + 1) * CW] if gi == 0
                          else xo_rest[:, xcol0 + c * CW:xcol0 + (c + 1) * CW])
                nc.gpsimd.dma_gather(
                    gt[:].rearrange("p (t e) -> p t e", e=EP),
                    gtab[g], idx_ap, NI, ni_reg, EP, queue_num=qn,
                )
                qn = (qn + 1) % NQ
                gtiles.append(gt)

                gt3 = ap3(gt)
                for t in range(TC):
                    nc.tensor.matmul(
                        out=ps[:], lhsT=ident_bf[:], rhs=gt3[:, t, 0:DIM],
                        start=(c == 0 and t == 0),
                        stop=(c == NCH - 1 and t == TC - 1),
                    )
                # mask (ACT per-token bias add) + L0 pair-max once the
                # sum has consumed the pair
                if c % 2 == 1 or c == NCH - 1:
                    for cc in ((c - 1, c) if c % 2 == 1 else (c,)):
                        clo = max(llo[g], cc * TC)
                        chi = min(mhi, (cc + 1) * TC)
                        if clo < chi and llo[g] < mhi and cc < nv:
                            gtf = gtiles[cc][:]
                            for i, tok in enumerate(range(clo, chi)):
                                ts = tok - cc * TC
                                sl = gtf[:, ts * EP:ts * EP + DIM]
                                bias_ap = ao_all[:, g * L + tok:
                                                 g * L + tok + 1]
                                if i % 2 == 0:
                                    nc.vector.tensor_scalar_add(
                                        out=sl, in0=sl, scalar1=bias_ap)
                                else:
                                    nc.scalar.activation(
                                        out=sl, in_=sl, func=ACT_F.Identity,
                                        bias=bias_ap, scale=1.0,
                                    )
                        if cc < nv and (cc % 2 == 1 or cc == nv - 1):
                            if cc % 2 == 1:
                                m = xpool.tile([P, TC * DIM], GDT,
                                               tag="tm", name="tm")
                                nc.vector.tensor_tensor(
                                    out=ap3c(m), in0=ap3(gtiles[cc - 1]),
                                    in1=ap3(gtiles[cc]), op=ALU.max)
                                push(max_stack, ap3c(m), ALU.max, xpool, "tm")
                            else:
                                push(max_stack, ap3(gtiles[cc]), ALU.max,
                                     xpool, "tm")

                if gi == 0 and c == W_AT:
                    emit_weight_dmas()
                if c == TAIL_A and tails:
                    tails[0][0]()
                if c == TAIL_B and tails:
                    tails.pop(0)[1]()
                if gi == G - 1 and c == LAST_A1:
                    last_hps = cur_tail[2]()

            if gi < G - 1:
                tails.append(cur_tail)
            else:
                cur_tail[3](last_hps)

    nc.compile()
    return nc


def _pack_idx16(idx_cg):
    """idx_cg: [P, L] group-local int indices. Returns [P, NCH*CW] int16
    (per chunk: 1024-entry list in i = t*128 + p order, 16-partition
    wrapped idxs[i%16, i//16], replicated to 128 partitions)."""
    out = np.empty((P, NCH * CW), dtype=np.int16)
    for c in range(NCH):
        lst = idx_cg[:, c * TC:(c + 1) * TC].T.reshape(-1)  # [NI] t-major
        wrapped = lst.reshape(CW, 16).T                     # [16, CW]
        out[:, c * CW:(c + 1) * CW] = np.tile(wrapped, (P // 16, 1))
    return out


def _prepare(inputs):
    emb_np = np.asarray(inputs["emb_table"], dtype=np.float32)
    x_np = np.ascontiguousarray(np.asarray(inputs["x"])).astype(np.int64)
    lengths = np.asarray(inputs["lengths"]).astype(np.int64)
    w1_np = np.ascontiguousarray(np.asarray(inputs["W1"], dtype=np.float32).astype(ml_dtypes.bfloat16))
    b1_np = np.ascontiguousarray(np.asarray(inputs["b1"], dtype=np.float32))
    w2_np = np.ascontiguousarray(np.asarray(inputs["W2"], dtype=np.float32).astype(ml_dtypes.bfloat16))
    b2_np = np.ascontiguousarray(np.asarray(inputs["b2"], dtype=np.float32))
    ident_np = np.eye(P, dtype=np.float32)

    # sort rows by length; rank r -> core r%8, slot r//8 so every core's
    # group g spans the same global length band (one SPMD program)
    order = np.argsort(lengths, kind="stable")
    rows_by_core = order.reshape(RPC, NCORES).T  # [8, 512]
    lens_cs = lengths[rows_by_core]              # [8, 512]
    lhi = tuple(int(lens_cs[:, g * P:(g + 1) * P].max()) for g in range(G))
    llo = tuple(int(lens_cs[:, g * P:(g + 1) * P].min()) for g in range(G))

    # per (core, group): compact table (unique rows) + int16 remapped idx
    uniqs, idx16s = [], []
    vg_req = 0
    for c in range(NCORES):
        rows = rows_by_core[c]
        for g in range(G):
            xg_blk = x_np[rows[g * P:(g + 1) * P]]          # [128, 200]
            uniq, inv = np.unique(xg_blk, return_inverse=True)
            assert len(uniq) < 32768, f"group table too large: {len(uniq)}"
            uniqs.append(uniq)
            idx16s.append(inv.reshape(P, L))
            vg_req = max(vg_req, len(uniq))
    vg = -(-vg_req // 16) * 16  # pad a little for alignment

    t_ar = np.arange(L)
    in_maps = []
    for c in range(NCORES):
        rows = rows_by_core[c]
        lc = lengths[rows]
        gtab = np.zeros((G, vg, EP), dtype=GNP)
        xg16 = np.empty((P, G * XGW), dtype=np.int16)
        for g in range(G):
            uniq = uniqs[c * G + g]
            gtab[g, :len(uniq), :DIM] = emb_np[uniq].astype(GNP)
            di = GROUP_ORDER.index(g)   # device order position
            xg16[:, di * XGW:(di + 1) * XGW] = _pack_idx16(idx16s[c * G + g])
        ac = np.where(t_ar[None, :] < lc[:, None], np.float32(0.0),
                      np.float32(NEG)).astype(np.float32).reshape(G, P, L)
        ao_pl = np.ascontiguousarray(ac.transpose(1, 0, 2).reshape(P, G * L))
        il = (1.0 / lc.astype(np.float64)).astype(np.float32).reshape(G, P)
        il_pg = np.ascontiguousarray(il.T)                  # [P, G]
        in_maps.append({
            "gtab": gtab, "xg": xg16,
            "aoff": ao_pl, "invlen": il_pg, "identD": ident_np,
            "w1": w1_np, "b1": b1_np, "w2": w2_np, "b2": b2_np,
        })
    return in_maps, rows_by_core, lhi, llo, vg


def run_with_results(inputs, trace=False, **kwargs):
    in_maps, rows_by_core, lhi, llo, vg = _prepare(inputs)
    key = (lhi, llo, vg)
    if key not in _BUILD_CACHE:
        _BUILD_CACHE[key] = _build(lhi, llo, vg)
    nc = _BUILD_CACHE[key]
    res = run_bass_kernel_spmd(
        nc, in_maps, core_ids=list(range(NCORES)), trace=trace, **kwargs
    )
    out = np.empty((B, OUT), np.float32)
    for c in range(NCORES):
        out[rows_by_core[c]] = np.asarray(res.results[c]["out_t"]).T
    return out, res


def kernel(**inputs) -> np.ndarray:
    out, _ = run_with_results(inputs, trace=False)
    return out


# revision 20
# speedup vs baseline: 1.0337x; 1.0337x over previous
"""Trainium2 Bass kernel for BaselineDNN pooling problem.

Per core (512 of 4096 batch rows, data-parallel across 8 cores):
  1. dma_gather (InstDMAGatherAnt ucode) fetches embedding rows from a
     per-group host-compacted table ([~23k unique rows, 768B stride])
     into [128 rows, 8 tokens, EP] SBUF tiles: index list position
     i = t*128 + p -> partition p, slot t. 1024 rows per instruction
     (int16 indices, 16-partition-wrapped, replicated x8), rotating over
     4 SWDGE queues so desc-gen overlaps DMA drain.
  2. PE identity-matmul accumulates the token sum into PSUM -> mean
     pool (x 1/len on ACT)
  3. DVE TT-max chain over valid chunks only (rows length-sorted on
     host so each 128-row group has a tight valid band). Boundary
     tokens are masked by per-token ACT bias-adds (+0 / -1e30): the
     Scalar engine is nearly idle, and a DVE broadcast add would run
     in slow 1x mode.
  4. PE transposes rep ([128,600] -> 5 x [120,128]) into rep_T
  5. MLP on PE (h_T = relu(W1_T @ rep_T + b1), out_T = W2_T @ h_T + b2)
  6. out_T [3,512] DMA'd out; host inverts the row permutation.

Scheduling: groups processed longest-first (3,2,1,0) so the final
group's tail is short; each group's tail work is EMITTED in two phases
during the next group's chunk stream (folds+mean+transposes, then MLP)
so the serial DVE fold chain doesn't head-of-line-block the pair ops
that recycle gather buffers. The identity matrix ships as a DRAM input
(generating it with Pool-ALU ucode would force the Pool engine through
a ucode-overlay switch costing ~12us before the first gather). First
group's indices load via SWDGE on the gpsimd engine itself; weight
DMAs are emitted mid-stream.

Self-contained: hardcodes all shapes from the problem spec.
"""

import numpy as np
from contextlib import ExitStack

import ml_dtypes

import concourse.bacc as bacc
import concourse.tile as tile
from concourse import mybir
from concourse.bass_utils import run_bass_kernel_spmd

VOCAB, DIM = 100000, 300
B, L = 4096, 200
HIDDEN, OUT = 1000, 3
NCORES = 8
P = 128
RPC = B // NCORES            # 512 rows per core
G = RPC // P                 # 4 groups of 128 rows
TC = 8                       # tokens per gather chunk (NI=1024 HW limit)
NCH = L // TC                # 25 chunks
NI = P * TC                  # 1024 indices per gather
CW = NI // 16                # idx columns per chunk (16-partition wrap)
NEG = -1.0e30
NQ = 4                       # SWDGE queues

EP = 384                     # padded row length (768B, 256B-divisible)

KC = 120                     # rep contraction chunk (600 = 5*120)
NKC = (2 * DIM) // KC        # 5
MJ = 125                     # hidden m-chunk (1000 = 8*125)
NJ = HIDDEN // MJ            # 8

GROUP_ORDER = (3, 2, 1, 0)   # longest valid window first
TAIL_A = 9                   # prev group: folds+mean+transposes
TAIL_B = 13                  # prev group: MLP
W_AT = 12                    # weight-DMA emission point in first group
LAST_A1 = 14                 # last group: early max-side emission point

F32 = mybir.dt.float32
BF16 = mybir.dt.bfloat16
GDT = BF16
GNP = ml_dtypes.bfloat16
I16 = mybir.dt.int16
AX = mybir.AxisListType
ALU = mybir.AluOpType
ACT_F = mybir.ActivationFunctionType

XGW = NCH * CW               # 1600 idx columns per group

_BUILD_CACHE = {}


def _build(lhi, llo, vg):
    """Emit the SPMD program. lhi/llo: per-group max/min valid length;
    vg: padded per-group compact-table row count (identical across cores
    by construction)."""
    nc = bacc.Bacc(
        "TRN2", target_bir_lowering=False, debug=False, enable_asserts=False,
        num_swdge_queues=NQ, dynamic_dma_scratch_size=32768,
    )
    gtab = nc.dram_tensor("gtab", [G, vg, EP], GDT, kind="ExternalInput")
    # index columns packed in device group order (GROUP_ORDER[0] first)
    xg = nc.dram_tensor("xg", [P, G * XGW], I16, kind="ExternalInput")
    aoff = nc.dram_tensor("aoff", [P, G * L], F32, kind="ExternalInput")
    invlen = nc.dram_tensor("invlen", [P, G], F32, kind="ExternalInput")
    identD = nc.dram_tensor("identD", [P, P], F32, kind="ExternalInput")
    w1 = nc.dram_tensor("w1", [2 * DIM, HIDDEN], BF16, kind="ExternalInput")
    b1 = nc.dram_tensor("b1", [HIDDEN], F32, kind="ExternalInput")
    w2 = nc.dram_tensor("w2", [HIDDEN, OUT], BF16, kind="ExternalInput")
    b2 = nc.dram_tensor("b2", [OUT], F32, kind="ExternalInput")
    out_t = nc.dram_tensor("out_t", [OUT, RPC], F32, kind="ExternalOutput")

    with tile.TileContext(nc) as tc, ExitStack() as ctx:
        persist = ctx.enter_context(tc.tile_pool(name="persist", bufs=1))
        gpool = ctx.enter_context(tc.tile_pool(name="gpool", bufs=10))
        xpool = ctx.enter_context(tc.tile_pool(name="xpool", bufs=7))
        mpool = ctx.enter_context(tc.tile_pool(name="mpool", bufs=2))
        spsum = ctx.enter_context(tc.tile_pool(name="spsum", bufs=2, space="PSUM"))
        ppool = ctx.enter_context(tc.tile_pool(name="ppool", bufs=2, space="PSUM"))
        hpool = ctx.enter_context(tc.tile_pool(name="hpool", bufs=2, space="PSUM"))
        opool = ctx.enter_context(tc.tile_pool(name="opool", bufs=1, space="PSUM"))

        # first group's indices via SWDGE on the gather engine itself —
        # no cross-engine semaphores before the first gather
        xo0 = persist.tile([P, XGW], I16, tag="xo0", name="xo0")
        nc.gpsimd.dma_start(xo0[:], xg[:, 0:XGW])

        ident = persist.tile([P, P], F32, tag="ident")
        nc.sync.dma_start(ident[:], identD[:])
        ident_bf = persist.tile([P, P], BF16, tag="ident_bf")
        nc.scalar.copy(out=ident_bf[:], in_=ident[:])

        # small per-group inputs (single consolidated DMAs)
        ao_all = persist.tile([P, G * L], F32, tag="ao_all", name="ao_all")
        nc.sync.dma_start(ao_all[:], aoff[:])
        il_all = persist.tile([P, G], F32, tag="il_all", name="il_all")
        nc.sync.dma_start(il_all[:], invlen[:])

        # weight tiles (DMAs emitted mid-stream at W_AT of first group)
        w1_t = [persist.tile([KC, HIDDEN], BF16, tag=f"w1_{k}", name=f"w1_{k}")
                for k in range(NKC)]
        w2_t = [persist.tile([MJ, OUT], BF16, tag=f"w2_{j}", name=f"w2_{j}")
                for j in range(NJ)]
        b1_t = [persist.tile([MJ, 1], F32, tag=f"b1_{j}", name=f"b1_{j}")
                for j in range(NJ)]
        b2_t = persist.tile([OUT, 1], F32, tag="b2")
        xo_rest = persist.tile([P, (G - 1) * XGW], I16, tag="xo_r", name="xo_r")

        def emit_weight_dmas():
            for k in range(NKC):
                nc.sync.dma_start(w1_t[k][:], w1[k * KC:(k + 1) * KC, :])
            for j in range(NJ):
                nc.sync.dma_start(w2_t[j][:], w2[j * MJ:(j + 1) * MJ, :])
                nc.sync.dma_start(b1_t[j][:], b1[j * MJ:(j + 1) * MJ, None])
            nc.sync.dma_start(b2_t[:], b2[:, None])
            nc.sync.dma_start(xo_rest[:], xg[:, XGW:])

        rep_t = [persist.tile([KC, RPC], BF16, tag=f"repT_{k}", name=f"repT_{k}")
                 for k in range(NKC)]
        h_t = [persist.tile([MJ, RPC], BF16, tag=f"hT_{j}", name=f"hT_{j}")
               for j in range(NJ)]
        ot_sb = persist.tile([OUT, RPC], F32, tag="ot", name="ot")

        def ap3(t):
            """[p, t, 0:DIM] view of a [P, TC*EP] chunk tile."""
            return t[:].rearrange("p (t e) -> p t e", e=EP)[:, :, 0:DIM]

        def ap3c(t):
            """[p, t, d] view of a [P, TC*DIM] contiguous tile."""
            return t[:].rearrange("p (t d) -> p t d", d=DIM)

        def fold(stack, op, pool, tag):
            # collapse remaining binary-counter levels into one root
            nodes = [stack[lv] for lv in sorted(stack)]
            stack.clear()
            while len(nodes) > 1:
                t = pool.tile([P, TC * DIM], GDT, tag=tag, name=tag)
                nc.vector.tensor_tensor(
                    out=ap3c(t), in0=nodes[0], in1=nodes[1], op=op)
                nodes = [ap3c(t)] + nodes[2:]
            return nodes[0]

        def push(stack, node, op, pool, tag):
            lv = 0
            while lv in stack:
                other = stack.pop(lv)
                t = pool.tile([P, TC * DIM], GDT, tag=tag, name=tag)
                nc.vector.tensor_tensor(
                    out=ap3c(t), in0=other, in1=node, op=op)
                node = ap3c(t)
                lv += 1
            stack[lv] = node

        def tfold_max(root, pool, out_f32):
            # fold token slots 8->4->2->1 into the f32 rep slice
            cur, nt = root, TC
            while nt > 2:
                nt //= 2
                t = pool.tile([P, nt * DIM], GDT, tag=f"tmf{nt}",
                              name="tmf", bufs=2)
                nc.vector.tensor_tensor(
                    out=t[:].rearrange("p (t d) -> p t d", d=DIM),
                    in0=cur[:, 0:nt, :], in1=cur[:, nt:2 * nt, :], op=ALU.max)
                cur = t[:].rearrange("p (t d) -> p t d", d=DIM)
            nc.vector.tensor_tensor(
                out=out_f32, in0=cur[:, 0:1, :],
                in1=cur[:, 1:2, :], op=ALU.max)

        ni_reg = NI
        qn = 0
        tails = []   # pending (phaseA, phaseB) closures

        def make_tail(g, max_stack, ps):
            gsl = slice(g * P, (g + 1) * P)
            il = il_all[:, g:g + 1]
            rep_sb = mpool.tile([P, 2 * DIM], F32, tag="rep_sb", name="rep_sb")

            def trans_rep(k):
                pt = ppool.tile([KC, P], F32, tag="pt", name="pt")
                nc.tensor.transpose(
                    out=pt[:], in_=rep_sb[:, k * KC:(k + 1) * KC],
                    identity=ident[:],
                )
                nc.scalar.copy(out=rep_t[k][:, gsl], in_=pt[:])

            def max_part():
                max_root = fold(max_stack, ALU.max, xpool, "tm")
                tfold_max(max_root, xpool,
                          rep_sb[:, DIM:2 * DIM].unsqueeze(1))

            def mean_part():
                nc.scalar.mul(rep_sb[:, 0:DIM], ps[:], il)

            def phase_a():
                max_part()
                mean_part()
                for k in range(NKC):
                    trans_rep(k)

            def mlp(hps, klist, start, stop):
                for j in range(NJ):
                    for ki, k in enumerate(klist):
                        nc.tensor.matmul(
                            out=hps[j][:],
                            lhsT=w1_t[k][:, j * MJ:(j + 1) * MJ],
                            rhs=rep_t[k][:, gsl],
                            start=(start and ki == 0),
                            stop=(stop and ki == len(klist) - 1),
                        )

            def finish(hps):
                for j in range(NJ):
                    nc.scalar.activation(
                        out=h_t[j][:, gsl], in_=hps[j][:], func=ACT_F.Relu,
                        bias=b1_t[j][:, 0:1], scale=1.0,
                    )
                op_ps = opool.tile([OUT, P], F32, tag="op", name="op", bufs=2)
                for j in range(NJ):
                    nc.tensor.matmul(
                        out=op_ps[:], lhsT=w2_t[j][:], rhs=h_t[j][:, gsl],
                        start=(j == 0), stop=(j == NJ - 1),
                    )
                nc.scalar.activation(
                    out=ot_sb[:, gsl], in_=op_ps[:], func=ACT_F.Identity,
                    bias=b2_t[:, 0:1], scale=1.0,
                )
                nc.sync.dma_start(out_t[:, gsl], ot_sb[:, gsl])

            def phase_b():
                hps = [hpool.tile([MJ, P], F32, tag="hp", name="hp")
                       for _ in range(NJ)]
                mlp(hps, list(range(NKC)), True, True)
                finish(hps)

            # split variant for the last group: max side early, mean late
            def early_max():
                # max dims are rep rows 300:600 -> k-chunks 3,4 pure max
                max_part()
                trans_rep(3)
                trans_rep(4)
                return None

            def late_mean(_):
                mean_part()
                trans_rep(0)
                trans_rep(1)
                trans_rep(2)
                phase_b()

            return phase_a, phase_b, early_max, late_mean

        for gi, g in enumerate(GROUP_ORDER):
            xcol0 = (gi - 1) * XGW

            nv = -(-lhi[g] // TC)          # chunks partaking in max pool
            mhi = min(nv * TC, L)          # mask window end (chunk-rounded)

            max_stack = {}
            ps = spsum.tile([P, DIM], F32, tag="ps", name="ps")
            cur_tail = make_tail(g, max_stack, ps)

            gtiles = []
            for c in range(NCH):
                gt = gpool.tile([P, TC * EP], GDT, tag="gt", name="gt")
                idx_ap = (xo0[:, c * CW:(c + 1) * CW] if gi == 0
                          else xo_rest[:, xcol0 + c * CW:xcol0 + (c + 1) * CW])
                nc.gpsimd.dma_gather(
                    gt[:].rearrange("p (t e) -> p t e", e=EP),
                    gtab[g], idx_ap, NI, ni_reg, EP, queue_num=qn,
                )
                qn = (qn + 1) % NQ
                gtiles.append(gt)

                gt3 = ap3(gt)
                for t in range(TC):
                    nc.tensor.matmul(
                        out=ps[:], lhsT=ident_bf[:], rhs=gt3[:, t, 0:DIM],
                        start=(c == 0 and t == 0),
                        stop=(c == NCH - 1 and t == TC - 1),
                    )
                # mask (ACT per-token bias add) + L0 pair-max once the
                # sum has consumed the pair
                if c % 2 == 1 or c == NCH - 1:
                    for cc in ((c - 1, c) if c % 2 == 1 else (c,)):
                        clo = max(llo[g], cc * TC)
                        chi = min(mhi, (cc + 1) * TC)
                        if clo < chi and llo[g] < mhi and cc < nv:
                            gtf = gtiles[cc][:]
                            for tok in range(clo, chi):
                                ts = tok - cc * TC
                                sl = gtf[:, ts * EP:ts * EP + DIM]
                                nc.scalar.activation(
                                    out=sl, in_=sl, func=ACT_F.Identity,
                                    bias=ao_all[:, g * L + tok:g * L + tok + 1],
                                    scale=1.0,
                                )
                        if cc < nv and (cc % 2 == 1 or cc == nv - 1):
                            if cc % 2 == 1:
                                m = xpool.tile([P, TC * DIM], GDT,
                                               tag="tm", name="tm")
                                nc.vector.tensor_tensor(
                                    out=ap3c(m), in0=ap3(gtiles[cc - 1]),
                                    in1=ap3(gtiles[cc]), op=ALU.max)
                                push(max_stack, ap3c(m), ALU.max, xpool, "tm")
                            else:
                                push(max_stack, ap3(gtiles[cc]), ALU.max,
                                     xpool, "tm")

                if gi == 0 and c == W_AT:
                    emit_weight_dmas()
                if c == TAIL_A and tails:
                    tails[0][0]()
                if c == TAIL_B and tails:
                    tails.pop(0)[1]()
                if gi == G - 1 and c == LAST_A1:
                    last_hps = cur_tail[2]()

            if gi < G - 1:
                tails.append(cur_tail)
            else:
                cur_tail[3](last_hps)

    nc.compile()
    return nc


def _pack_idx16(idx_cg):
    """idx_cg: [P, L] group-local int indices. Returns [P, NCH*CW] int16
    (per chunk: 1024-entry list in i = t*128 + p order, 16-partition
    wrapped idxs[i%16, i//16], replicated to 128 partitions)."""
    out = np.empty((P, NCH * CW), dtype=np.int16)
    for c in range(NCH):
        lst = idx_cg[:, c * TC:(c + 1) * TC].T.reshape(-1)  # [NI] t-major
        wrapped = lst.reshape(CW, 16).T                     # [16, CW]
        out[:, c * CW:(c + 1) * CW] = np.tile(wrapped, (P // 16, 1))
    return out


def _prepare(inputs):
    emb_np = np.asarray(inputs["emb_table"], dtype=np.float32)
    x_np = np.ascontiguousarray(np.asarray(inputs["x"])).astype(np.int64)
    lengths = np.asarray(inputs["lengths"]).astype(np.int64)
    w1_np = np.ascontiguousarray(np.asarray(inputs["W1"], dtype=np.float32).astype(ml_dtypes.bfloat16))
    b1_np = np.ascontiguousarray(np.asarray(inputs["b1"], dtype=np.float32))
    w2_np = np.ascontiguousarray(np.asarray(inputs["W2"], dtype=np.float32).astype(ml_dtypes.bfloat16))
    b2_np = np.ascontiguousarray(np.asarray(inputs["b2"], dtype=np.float32))
    ident_np = np.eye(P, dtype=np.float32)

    # sort rows by length; rank r -> core r%8, slot r//8 so every core's
    # group g spans the same global length band (one SPMD program)
    order = np.argsort(lengths, kind="stable")
    rows_by_core = order.reshape(RPC, NCORES).T  # [8, 512]
    lens_cs = lengths[rows_by_core]              # [8, 512]
    lhi = tuple(int(lens_cs[:, g * P:(g + 1) * P].max()) for g in range(G))
    llo = tuple(int(lens_cs[:, g * P:(g + 1) * P].min()) for g in range(G))

    # per (core, group): compact table (unique rows) + int16 remapped idx
    uniqs, idx16s = [], []
    vg_req = 0
    for c in range(NCORES):
        rows = rows_by_core[c]
        for g in range(G):
            xg_blk = x_np[rows[g * P:(g + 1) * P]]          # [128, 200]
            uniq, inv = np.unique(xg_blk, return_inverse=True)
            assert len(uniq) < 32768, f"group table too large: {len(uniq)}"
            uniqs.append(uniq)
            idx16s.append(inv.reshape(P, L))
            vg_req = max(vg_req, len(uniq))
    vg = -(-vg_req // 16) * 16  # pad a little for alignment

    t_ar = np.arange(L)
    in_maps = []
    for c in range(NCORES):
        rows = rows_by_core[c]
        lc = lengths[rows]
        gtab = np.zeros((G, vg, EP), dtype=GNP)
        xg16 = np.empty((P, G * XGW), dtype=np.int16)
        for g in range(G):
            uniq = uniqs[c * G + g]
            gtab[g, :len(uniq), :DIM] = emb_np[uniq].astype(GNP)
            di = GROUP_ORDER.index(g)   # device order position
            xg16[:, di * XGW:(di + 1) * XGW] = _pack_idx16(idx16s[c * G + g])
        ac = np.where(t_ar[None, :] < lc[:, None], np.float32(0.0),
                      np.float32(NEG)).astype(np.float32).reshape(G, P, L)
        ao_pl = np.ascontiguousarray(ac.transpose(1, 0, 2).reshape(P, G * L))
        il = (1.0 / lc.astype(np.float64)).astype(np.float32).reshape(G, P)
        il_pg = np.ascontiguousarray(il.T)                  # [P, G]
        in_maps.append({
            "gtab": gtab, "xg": xg16,
            "aoff": ao_pl, "invlen": il_pg, "identD": ident_np,
            "w1": w1_np, "b1": b1_np, "w2": w2_np, "b2": b2_np,
        })
    return in_maps, rows_by_core, lhi, llo, vg


def run_with_results(inputs, trace=False, **kwargs):
    in_maps, rows_by_core, lhi, llo, vg = _prepare(inputs)
    key = (lhi, llo, vg)
    if key not in _BUILD_CACHE:
        _BUILD_CACHE[key] = _build(lhi, llo, vg)
    nc = _BUILD_CACHE[key]
    res = run_bass_kernel_spmd(
        nc, in_maps, core_ids=list(range(NCORES)), trace=trace, **kwargs
    )
    out = np.empty((B, OUT), np.float32)
    for c in range(NCORES):
        out[rows_by_core[c]] = np.asarray(res.results[c]["out_t"]).T
    return out, res


def kernel(**inputs) -> np.ndarray:
    out, _ = run_with_results(inputs, trace=False)
    return out


# revision 32
# speedup vs baseline: 1.2819x; 1.2401x over previous
"""Trainium2 Bass kernel for BaselineDNN pooling problem.

Per core (512 of 4096 batch rows, data-parallel across 8 cores):
  1. dma_gather (InstDMAGatherAnt ucode) fetches embedding rows from a
     per-group host-compacted table ([~23k unique rows, 768B stride])
     into [128 rows, 8 tokens, EP] SBUF tiles: index list position
     i = t*128 + p -> partition p, slot t. 1024 rows per instruction
     (int16 indices, 16-partition-wrapped, replicated x8), rotating over
     4 SWDGE queues so desc-gen overlaps DMA drain.
  2. PE identity-matmul accumulates the token sum into PSUM -> mean
     pool (x 1/len on ACT)
  3. DVE TT-max chain over valid chunks only (rows length-sorted on
     host so each 128-row group has a tight valid band). Boundary
     tokens are masked by per-token ACT bias-adds (+0 / -1e30): the
     Scalar engine is nearly idle, and a DVE broadcast add would run
     in slow 1x mode.
  4. PE transposes rep ([128,600] -> 5 x [120,128]) into rep_T
  5. MLP on PE (h_T = relu(W1_T @ rep_T + b1), out_T = W2_T @ h_T + b2)
  6. out_T [3,512] DMA'd out; host inverts the row permutation.

Scheduling: groups processed longest-first (3,2,1,0) so the final
group's tail is short; each group's tail work is EMITTED in two phases
during the next group's chunk stream (folds+mean+transposes, then MLP)
so the serial DVE fold chain doesn't head-of-line-block the pair ops
that recycle gather buffers. The identity matrix ships as a DRAM input
(generating it with Pool-ALU ucode would force the Pool engine through
a ucode-overlay switch costing ~12us before the first gather). First
group's indices load via SWDGE on the gpsimd engine itself; weight
DMAs are emitted mid-stream.

Self-contained: hardcodes all shapes from the problem spec.
"""

import numpy as np
from contextlib import ExitStack

import ml_dtypes

import concourse.bacc as bacc
import concourse.tile as tile
from concourse import mybir
from concourse.bass_utils import run_bass_kernel_spmd

VOCAB, DIM = 100000, 300
B, L = 4096, 200
HIDDEN, OUT = 1000, 3
NCORES = 8
P = 128
RPC = B // NCORES            # 512 rows per core
G = RPC // P                 # 4 groups of 128 rows
TC = 8                       # tokens per gather chunk (NI=1024 HW limit)
NCH = L // TC                # 25 chunks
NI = P * TC                  # 1024 indices per gather
CW = NI // 16                # idx columns per chunk (16-partition wrap)
NEG = -1.0e30
NQ = 4                       # SWDGE queues

EP = 384                     # padded row length (768B, 256B-divisible)
EP8 = 512                    # fp8 row length (512B stride) for sum-only chunks
FP8_GROUPS = (1, 2)          # groups whose padding chunks gather in fp8

KC = 120                     # rep contraction chunk (600 = 5*120)
NKC = (2 * DIM) // KC        # 5
MJ = 125                     # hidden m-chunk (1000 = 8*125)
NJ = HIDDEN // MJ            # 8

GROUP_ORDER = (3, 2, 1, 0)   # longest valid window first
TAIL_A = 9                   # prev group: folds+mean+transposes
TAIL_B = 13                  # prev group: MLP
W_AT = 4                     # weight-DMA emission point in first group
LAST_A1 = 14                 # last group: early max-side emission point

F32 = mybir.dt.float32
F8 = mybir.dt.float8e4
BF16 = mybir.dt.bfloat16
GDT = BF16
GNP = ml_dtypes.bfloat16
I16 = mybir.dt.int16
AX = mybir.AxisListType
ALU = mybir.AluOpType
ACT_F = mybir.ActivationFunctionType

XGW = NCH * CW               # 1600 idx columns per group

_BUILD_CACHE = {}


def _build(lhi, llo, vg):
    """Emit the SPMD program. lhi/llo: per-group max/min valid length;
    vg: padded per-group compact-table row count (identical across cores
    by construction)."""
    nc = bacc.Bacc(
        "TRN2", target_bir_lowering=False, debug=False, enable_asserts=False,
        num_swdge_queues=NQ, dynamic_dma_scratch_size=32768,
    )
    gtab = nc.dram_tensor("gtab", [G, vg, EP], GDT, kind="ExternalInput")
    gtab8 = nc.dram_tensor("gtab8", [len(FP8_GROUPS), vg, EP8], F8,
                           kind="ExternalInput")
    # index columns packed in device group order (GROUP_ORDER[0] first)
    xg = nc.dram_tensor("xg", [P, G * XGW], I16, kind="ExternalInput")
    aoff = nc.dram_tensor("aoff", [P, G * L], F32, kind="ExternalInput")
    invlen = nc.dram_tensor("invlen", [P, G], F32, kind="ExternalInput")
    identD = nc.dram_tensor("identD", [P, P], F32, kind="ExternalInput")
    w1 = nc.dram_tensor("w1", [KC, NKC * HIDDEN], BF16, kind="ExternalInput")
    b1 = nc.dram_tensor("b1", [MJ, NJ], F32, kind="ExternalInput")
    w2 = nc.dram_tensor("w2", [MJ, NJ * OUT], BF16, kind="ExternalInput")
    b2 = nc.dram_tensor("b2", [OUT], F32, kind="ExternalInput")
    out_t = nc.dram_tensor("out_t", [OUT, RPC], F32, kind="ExternalOutput")

    with tile.TileContext(nc) as tc, ExitStack() as ctx:
        persist = ctx.enter_context(tc.tile_pool(name="persist", bufs=1))
        gpool = ctx.enter_context(tc.tile_pool(name="gpool", bufs=12))
        xpool = ctx.enter_context(tc.tile_pool(name="xpool", bufs=5))
        mpool = ctx.enter_context(tc.tile_pool(name="mpool", bufs=2))
        spsum = ctx.enter_context(tc.tile_pool(name="spsum", bufs=2, space="PSUM"))
        ppool = ctx.enter_context(tc.tile_pool(name="ppool", bufs=2, space="PSUM"))
        hpool = ctx.enter_context(tc.tile_pool(name="hpool", bufs=2, space="PSUM"))
        opool = ctx.enter_context(tc.tile_pool(name="opool", bufs=1, space="PSUM"))

        # first group's indices: very first HWDGE DMA so the Pool engine
        # has nothing before its first gather (the one-time extended-ucode
        # load starts at dispatch of that gather)
        xo0 = persist.tile([P, XGW], I16, tag="xo0", name="xo0")
        nc.sync.dma_start(xo0[:], xg[:, 0:XGW])

        ident = persist.tile([P, P], F32, tag="ident")
        nc.sync.dma_start(ident[:], identD[:])
        ident_bf = persist.tile([P, P], BF16, tag="ident_bf")
        nc.scalar.copy(out=ident_bf[:], in_=ident[:])
        ident_f8 = persist.tile([P, P], F8, tag="ident_f8")
        nc.scalar.copy(out=ident_f8[:], in_=ident[:])

        # small per-group inputs (single consolidated DMAs)
        ao_all = persist.tile([P, G * L], F32, tag="ao_all", name="ao_all")
        nc.sync.dma_start(ao_all[:], aoff[:])
        il_all = persist.tile([P, G], F32, tag="il_all", name="il_all")
        nc.sync.dma_start(il_all[:], invlen[:])

        # weight tiles (4 packed DMAs emitted at W_AT of first group,
        # landing in the startup dead window before gathers saturate HBM)
        w1_all = persist.tile([KC, NKC * HIDDEN], BF16, tag="w1a", name="w1a")
        w2_all = persist.tile([MJ, NJ * OUT], BF16, tag="w2a", name="w2a")
        b1_all = persist.tile([MJ, NJ], F32, tag="b1a", name="b1a")
        b2_t = persist.tile([OUT, 1], F32, tag="b2")
        xo_rest = persist.tile([P, (G - 1) * XGW], I16, tag="xo_r", name="xo_r")
        w1_t = [w1_all[:, k * HIDDEN:(k + 1) * HIDDEN] for k in range(NKC)]

        def emit_weight_dmas():
            nc.sync.dma_start(w1_all[:], w1[:])
            nc.sync.dma_start(w2_all[:], w2[:])
            nc.sync.dma_start(b1_all[:], b1[:])
            nc.sync.dma_start(b2_t[:], b2[:, None])
            nc.sync.dma_start(xo_rest[:], xg[:, XGW:])

        rep_t = [persist.tile([KC, RPC], BF16, tag=f"repT_{k}", name=f"repT_{k}")
                 for k in range(NKC)]
        hpart = persist.tile([MJ, NJ * P], F32, tag="hpart", name="hpart")
        h_t = [persist.tile([MJ, RPC], BF16, tag=f"hT_{j}", name=f"hT_{j}")
               for j in range(NJ)]
        ot_sb = persist.tile([OUT, RPC], F32, tag="ot", name="ot")

        def ap3(t):
            """[p, t, 0:DIM] view of a [P, TC*EP] chunk tile."""
            return t[:].rearrange("p (t e) -> p t e", e=EP)[:, :, 0:DIM]

        def apw3(t):
            """[p, t, e] full view of a [P, TC*EP] chunk tile."""
            return t[:].rearrange("p (t e) -> p t e", e=EP)

        mask_alt = [0]

        def ap3c(t):
            """[p, t, d] view of a [P, TC*DIM] contiguous tile."""
            return t[:].rearrange("p (t d) -> p t d", d=DIM)

        def push(stack, node, op, pool, tag):
            # running accumulator: one DVE op per pushed node, no bursts
            if "acc" in stack:
                t = pool.tile([P, TC * DIM], GDT, tag=tag, name=tag)
                nc.vector.tensor_tensor(
                    out=ap3c(t), in0=stack["acc"], in1=node, op=op)
                stack["acc"] = ap3c(t)
            else:
                stack["acc"] = node

        def fold(stack, op, pool, tag):
            return stack.pop("acc")

        def tfold_max(root, pool, out_f32):
            # fold token slots 8->4->2->1 into the f32 rep slice
            cur, nt = root, TC
            while nt > 2:
                nt //= 2
                t = pool.tile([P, nt * DIM], GDT, tag=f"tmf{nt}",
                              name="tmf", bufs=2)
                nc.vector.tensor_tensor(
                    out=t[:].rearrange("p (t d) -> p t d", d=DIM),
                    in0=cur[:, 0:nt, :], in1=cur[:, nt:2 * nt, :], op=ALU.max)
                cur = t[:].rearrange("p (t d) -> p t d", d=DIM)
            nc.vector.tensor_tensor(
                out=out_f32, in0=cur[:, 0:1, :],
                in1=cur[:, 1:2, :], op=ALU.max)

        ni_reg = NI
        qn = 0
        tails = []   # pending (phaseA, phaseB) closures

        def make_tail(g, max_stack, ps):
            gsl = slice(g * P, (g + 1) * P)
            il = il_all[:, g:g + 1]
            rep_sb = mpool.tile([P, 2 * DIM], F32, tag="rep_sb", name="rep_sb")

            def trans_rep(k):
                pt = ppool.tile([KC, P], F32, tag="pt", name="pt")
                nc.tensor.transpose(
                    out=pt[:], in_=rep_sb[:, k * KC:(k + 1) * KC],
                    identity=ident[:],
                )
                nc.vector.tensor_scalar_add(out=rep_t[k][:, gsl], in0=pt[:], scalar1=0.0)

            def max_part():
                max_root = fold(max_stack, ALU.max, xpool, "tm")
                tfold_max(max_root, xpool,
                          rep_sb[:, DIM:2 * DIM].unsqueeze(1))

            def mean_part():
                nc.scalar.mul(rep_sb[:, 0:DIM], ps[:], il)

            def phase_a():
                max_part()
                mean_part()
                for k in range(NKC):
                    trans_rep(k)

            def mlp(hps, klist, start, stop):
                for j in range(NJ):
                    for ki, k in enumerate(klist):
                        nc.tensor.matmul(
                            out=hps[j][:],
                            lhsT=w1_all[:, k * HIDDEN + j * MJ:
                                        k * HIDDEN + (j + 1) * MJ],
                            rhs=rep_t[k][:, gsl],
                            start=(start and ki == 0),
                            stop=(stop and ki == len(klist) - 1),
                        )

            def finish(hps):
                for j in range(NJ):
                    nc.scalar.activation(
                        out=h_t[j][:, gsl], in_=hps[j][:], func=ACT_F.Relu,
                        bias=b1_all[:, j:j + 1], scale=1.0,
                    )
                op_ps = opool.tile([OUT, P], F32, tag="op", name="op", bufs=2)
                for j in range(NJ):
                    nc.tensor.matmul(
                        out=op_ps[:], lhsT=w2_all[:, j * OUT:(j + 1) * OUT],
                        rhs=h_t[j][:, gsl],
                        start=(j == 0), stop=(j == NJ - 1),
                    )
                nc.scalar.activation(
                    out=ot_sb[:, gsl], in_=op_ps[:], func=ACT_F.Identity,
                    bias=b2_t[:, 0:1], scale=1.0,
                )
                nc.sync.dma_start(out_t[:, gsl], ot_sb[:, gsl])

            def phase_b():
                hps = [hpool.tile([MJ, P], F32, tag="hp", name="hp")
                       for _ in range(NJ)]
                mlp(hps, list(range(NKC)), True, True)
                finish(hps)

            # split variant for the last group: max side early, mean late.
            # The max-dim MLP contribution (k=3,4) is computed mid-stream
            # into bf16 SBUF; the tail only runs k=0,1,2 + add + relu.
            def early_max():
                max_part()
                trans_rep(3)
                trans_rep(4)
                for j in range(NJ):
                    hp = hpool.tile([MJ, P], F32, tag="hp", name="hp")
                    for ki, k in enumerate((3, 4)):
                        nc.tensor.matmul(
                            out=hp[:],
                            lhsT=w1_all[:, k * HIDDEN + j * MJ:
                                        k * HIDDEN + (j + 1) * MJ],
                            rhs=rep_t[k][:, gsl],
                            start=(ki == 0), stop=(ki == 1),
                        )
                    nc.scalar.copy(
                        out=hpart[:, j * P:(j + 1) * P], in_=hp[:])
                return None

            def late_mean(_):
                mean_part()
                trans_rep(0)
                trans_rep(1)
                trans_rep(2)
                for j in range(NJ):
                    hp = hpool.tile([MJ, P], F32, tag="hp", name="hp")
                    for ki, k in enumerate((0, 1, 2)):
                        nc.tensor.matmul(
                            out=hp[:],
                            lhsT=w1_all[:, k * HIDDEN + j * MJ:
                                        k * HIDDEN + (j + 1) * MJ],
                            rhs=rep_t[k][:, gsl],
                            start=(ki == 0), stop=(ki == 2),
                        )
                    hsum = mpool.tile([MJ, P], BF16, tag="hsum", name="hsum")
                    nc.vector.tensor_tensor(
                        out=hsum[:], in0=hp[:],
                        in1=hpart[:, j * P:(j + 1) * P], op=ALU.add)
                    nc.scalar.activation(
                        out=h_t[j][:, gsl], in_=hsum[:], func=ACT_F.Relu,
                        bias=b1_all[:, j:j + 1], scale=1.0,
                    )
                op_ps = opool.tile([OUT, P], F32, tag="op", name="op", bufs=2)
                for j in range(NJ):
                    nc.tensor.matmul(
                        out=op_ps[:], lhsT=w2_all[:, j * OUT:(j + 1) * OUT],
                        rhs=h_t[j][:, gsl],
                        start=(j == 0), stop=(j == NJ - 1),
                    )
                nc.scalar.activation(
                    out=ot_sb[:, gsl], in_=op_ps[:], func=ACT_F.Identity,
                    bias=b2_t[:, 0:1], scale=1.0,
                )
                nc.sync.dma_start(out_t[:, gsl], ot_sb[:, gsl])

            return phase_a, phase_b, early_max, late_mean

        for gi, g in enumerate(GROUP_ORDER):
            xcol0 = (gi - 1) * XGW

            nv = -(-lhi[g] // TC)          # chunks partaking in max pool
            mhi = min(nv * TC, L)          # mask window end (chunk-rounded)

            max_stack = {}
            ps = spsum.tile([P, DIM], F32, tag="ps", name="ps")
            cur_tail = make_tail(g, max_stack, ps)

            # chunk processing order: spread masked chunks (heavy ACT
            # bias-add bursts) between unmasked ones so the Scalar engine
            # keeps pace with the gather stream; sum and max pooling are
            # order-invariant. Last group: masked every 2nd slot so its
            # max tree still completes early.
            masked = [c for c in range(nv)
                      if max(llo[g], c * TC) < min(mhi, (c + 1) * TC)
                      and llo[g] < mhi]
            others = ([c for c in range(nv) if c not in masked]
                      + [c for c in range(NCH) if c >= nv])
            step = 2 if gi == G - 1 else 3
            seq = []
            mi = oi = 0
            for si in range(NCH):
                if mi < len(masked) and si % step == step - 1:
                    seq.append(masked[mi]); mi += 1
                elif oi < len(others):
                    seq.append(others[oi]); oi += 1
                else:
                    seq.append(masked[mi]); mi += 1

            gtiles = {}
            pend = []
            for si, c in enumerate(seq):
                use_f8 = (g in FP8_GROUPS and c >= nv)
                gt = gpool.tile([P, TC * EP], GDT, tag="gt", name="gt")
                idx_ap = (xo0[:, c * CW:(c + 1) * CW] if gi == 0
                          else xo_rest[:, xcol0 + c * CW:xcol0 + (c + 1) * CW])
                if use_f8:
                    g8v = gt[:].bitcast(F8)[:, 0:TC * EP8]
                    nc.gpsimd.dma_gather(
                        g8v.rearrange("p (t e) -> p t e", e=EP8),
                        gtab8[FP8_GROUPS.index(g)], idx_ap, NI, ni_reg, EP8,
                        queue_num=qn,
                    )
                else:
                    nc.gpsimd.dma_gather(
                        gt[:].rearrange("p (t e) -> p t e", e=EP),
                        gtab[g], idx_ap, NI, ni_reg, EP, queue_num=qn,
                    )
                qn = (qn + 1) % NQ
                gtiles[c] = gt

                if use_f8:
                    gt3 = gt[:].bitcast(F8)[:, 0:TC * EP8].rearrange(
                        "p (t e) -> p t e", e=EP8)
                    lhs_i = ident_f8
                else:
                    gt3 = ap3(gt)
                    lhs_i = ident_bf
                for t in range(TC):
                    nc.tensor.matmul(
                        out=ps[:], lhsT=lhs_i[:], rhs=gt3[:, t, 0:DIM],
                        start=(si == 0 and t == 0),
                        stop=(si == NCH - 1 and t == TC - 1),
                        skip_group_check=True,
                    )
                # mask right after the sum reads; alternate chunks between
                # DVE (one broadcast tensor_tensor, 1x mode) and ACT (one
                # bias-add per token) so neither engine's in-order queue
                # becomes the slot-recycling bottleneck
                clo = max(llo[g], c * TC)
                chi = min(mhi, (c + 1) * TC)
                if clo < chi and llo[g] < mhi and c < nv:
                    mask_alt[0] = (mask_alt[0] + 1) % 3
                    if mask_alt[0] == 0:
                        n = chi - clo
                        sl = apw3(gt)[:, clo - c * TC:chi - c * TC, 0:DIM]
                        ab = ao_all[:, g * L + clo:g * L + chi].unsqueeze(
                            2).broadcast_to([P, n, DIM])
                        nc.vector.tensor_tensor(
                            out=sl, in0=sl, in1=ab, op=ALU.add)
                    else:
                        gtf = gt[:]
                        for tok in range(clo, chi):
                            ts = tok - c * TC
                            sl = gtf[:, ts * EP:ts * EP + DIM]
                            nc.scalar.activation(
                                out=sl, in_=sl, func=ACT_F.Identity,
                                bias=ao_all[:, g * L + tok:g * L + tok + 1],
                                scale=1.0,
                            )
                # L0 pair-max between valid chunks in arrival order
                if c < nv:
                    pend.append(c)
                    if len(pend) == 2:
                        a, b = pend
                        pend = []
                        m = xpool.tile([P, TC * DIM], GDT, tag="tm", name="tm")
                        nc.vector.tensor_tensor(
                            out=ap3c(m), in0=ap3(gtiles[a]),
                            in1=ap3(gtiles[b]), op=ALU.max)
                        push(max_stack, ap3c(m), ALU.max, xpool, "tm")

                if gi == 0 and si == W_AT:
                    emit_weight_dmas()
                if si == TAIL_A and tails:
                    tails[0][0]()
                if si == TAIL_B and tails:
                    tails.pop(0)[1]()
                if gi == G - 1 and si == LAST_A1:
                    if pend:
                        push(max_stack, ap3(gtiles[pend.pop()]), ALU.max,
                             xpool, "tm")
                    last_hps = cur_tail[2]()

            if pend:
                push(max_stack, ap3(gtiles[pend.pop()]), ALU.max,
                     xpool, "tm")

            if gi < G - 1:
                tails.append(cur_tail)
            else:
                cur_tail[3](last_hps)

    nc.compile()
    return nc


def _pack_idx16(idx_cg):
    """idx_cg: [P, L] group-local int indices. Returns [P, NCH*CW] int16
    (per chunk: 1024-entry list in i = t*128 + p order, 16-partition
    wrapped idxs[i%16, i//16], replicated to 128 partitions)."""
    out = np.empty((P, NCH * CW), dtype=np.int16)
    for c in range(NCH):
        lst = idx_cg[:, c * TC:(c + 1) * TC].T.reshape(-1)  # [NI] t-major
        wrapped = lst.reshape(CW, 16).T                     # [16, CW]
        out[:, c * CW:(c + 1) * CW] = np.tile(wrapped, (P // 16, 1))
    return out


def _prepare(inputs):
    emb_np = np.asarray(inputs["emb_table"], dtype=np.float32)
    x_np = np.ascontiguousarray(np.asarray(inputs["x"])).astype(np.int64)
    lengths = np.asarray(inputs["lengths"]).astype(np.int64)
    w1f = np.asarray(inputs["W1"], dtype=np.float32).astype(ml_dtypes.bfloat16)
    # pack [600,1000] -> [120, 5*1000] (k-chunk-major columns)
    w1_np = np.ascontiguousarray(
        w1f.reshape(NKC, KC, HIDDEN).transpose(1, 0, 2).reshape(KC, NKC * HIDDEN))
    b1f = np.asarray(inputs["b1"], dtype=np.float32)
    b1_np = np.ascontiguousarray(b1f.reshape(NJ, MJ).T)          # [125, 8]
    w2f = np.asarray(inputs["W2"], dtype=np.float32).astype(ml_dtypes.bfloat16)
    w2_np = np.ascontiguousarray(
        w2f.reshape(NJ, MJ, OUT).transpose(1, 0, 2).reshape(MJ, NJ * OUT))
    b2_np = np.ascontiguousarray(np.asarray(inputs["b2"], dtype=np.float32))
    ident_np = np.eye(P, dtype=np.float32)

    # sort rows by length; rank r -> core r%8, slot r//8 so every core's
    # group g spans the same global length band (one SPMD program)
    order = np.argsort(lengths, kind="stable")
    rows_by_core = order.reshape(RPC, NCORES).T  # [8, 512]
    lens_cs = lengths[rows_by_core]              # [8, 512]
    lhi = tuple(int(lens_cs[:, g * P:(g + 1) * P].max()) for g in range(G))
    llo = tuple(int(lens_cs[:, g * P:(g + 1) * P].min()) for g in range(G))

    # per (core, group): compact table (unique rows) + int16 remapped idx
    uniqs, idx16s = [], []
    vg_req = 0
    for c in range(NCORES):
        rows = rows_by_core[c]
        for g in range(G):
            xg_blk = x_np[rows[g * P:(g + 1) * P]]          # [128, 200]
            uniq, inv = np.unique(xg_blk, return_inverse=True)
            assert len(uniq) < 32768, f"group table too large: {len(uniq)}"
            uniqs.append(uniq)
            idx16s.append(inv.reshape(P, L))
            vg_req = max(vg_req, len(uniq))
    vg = -(-vg_req // 16) * 16  # pad a little for alignment

    t_ar = np.arange(L)
    in_maps = []
    for c in range(NCORES):
        rows = rows_by_core[c]
        lc = lengths[rows]
        gtab = np.zeros((G, vg, EP), dtype=GNP)
        gtab8 = np.zeros((len(FP8_GROUPS), vg, EP8), dtype=ml_dtypes.float8_e4m3)
        xg16 = np.empty((P, G * XGW), dtype=np.int16)
        for g in range(G):
            uniq = uniqs[c * G + g]
            gtab[g, :len(uniq), :DIM] = emb_np[uniq].astype(GNP)
            if g in FP8_GROUPS:
                gtab8[FP8_GROUPS.index(g), :len(uniq), :DIM] = (
                    emb_np[uniq].astype(ml_dtypes.float8_e4m3))
            di = GROUP_ORDER.index(g)   # device order position
            xg16[:, di * XGW:(di + 1) * XGW] = _pack_idx16(idx16s[c * G + g])
        ac = np.where(t_ar[None, :] < lc[:, None], np.float32(0.0),
                      np.float32(NEG)).astype(np.float32).reshape(G, P, L)
        ao_pl = np.ascontiguousarray(ac.transpose(1, 0, 2).reshape(P, G * L))
        il = (1.0 / lc.astype(np.float64)).astype(np.float32).reshape(G, P)
        il_pg = np.ascontiguousarray(il.T)                  # [P, G]
        in_maps.append({
            "gtab": gtab, "gtab8": gtab8, "xg": xg16,
            "aoff": ao_pl, "invlen": il_pg, "identD": ident_np,
            "w1": w1_np, "b1": b1_np, "w2": w2_np, "b2": b2_np,
        })
    return in_maps, rows_by_core, lhi, llo, vg


def run_with_results(inputs, trace=False, **kwargs):
    in_maps, rows_by_core, lhi, llo, vg = _prepare(inputs)
    key = (lhi, llo, vg)
    if key not in _BUILD_CACHE:
        _BUILD_CACHE[key] = _build(lhi, llo, vg)
    nc = _BUILD_CACHE[key]
    res = run_bass_kernel_spmd(
        nc, in_maps, core_ids=list(range(NCORES)), trace=trace, **kwargs
    )
    out = np.empty((B, OUT), np.float32)
    for c in range(NCORES):
        out[rows_by_core[c]] = np.asarray(res.results[c]["out_t"]).T
    return out, res


def kernel(**inputs) -> np.ndarray:
    out, _ = run_with_results(inputs, trace=False)
    return out
